# revision 7
# baseline (speedup 1.0000x reference)
"""LocallyConnected1d Trainium2 kernel.

Problem: out[b, oc, w] = sum_{ic,k} xp[b, ic, w+k] * W[w, oc, ic, k] + bias[oc, w]
  x: (32, 64, 2048) f32, weights: (2048, 64, 64, 3) f32, bias: (64, 2048) f32
  out: (32, 64, 2048) f32.  xp = x padded by 1 on both sides of the last axis.

Sharding: output_width (2048) is split into 8 contiguous chunks of 256, one per
NeuronCore.  Weights dominate the traffic (100 MB) and are fully sharded this
way (~13 MB/core); x is sent with a 2-column halo.

Per-core compute is organized per INPUT column c (local 0..257; xp column
ws+c).  Column c is consumed by three output positions: out[c] (tap k=0),
out[c-1] (tap k=1), out[c-2] (tap k=2).  Two matmuls per column, sharing the
single rhs X[:, c] = xp[:, ws+c].T (shape [ic, b]):

  mm1: lhsT = [Wk0(c) | Wk1(c-1)]  [64ic, 128]  -> psum slot c    rows 0:128
       (rows 0:64 = k0 contribution to out[c], rows 64:128 = k1 to out[c-1];
        the M=128 packing halves LDWEIGHTS rows vs. two separate matmuls)
  mm2: lhsT = [Wk2(c-2); bias row] [65,   64]   -> psum slot c-2  rows 0:64
       (PSUM-accumulated onto mm1's k0 rows; bias folded via lhsT row 64
        against the constant-one partition 64 of X)

After mm2(c+2), psum slot c rows 0:64 hold k0+k2+bias of out[c]; rows 64:128
of slot c+1 hold k1 of out[c].  One DVE add combines them into SBUF.

Host-side prep packs wm1/wm2/xc in exactly these layouts (zeros where a tap
would cross the core boundary; neighbouring cores own those outputs).
"""

import numpy as np

import concourse.bacc as bacc
import concourse.mybir as mybir
import concourse.tile as tile
from concourse.bass_utils import run_bass_kernel_spmd

B, IC, OC, KS, W = 32, 64, 64, 3, 2048
NCORES = 8
OWC = W // NCORES   # 256 output positions per core
NCOL = OWC + 2      # 258 input columns (2-col halo)
CB = 16             # columns per block; psum tile = [128, CB*32] = 1 bank
NBLK = (NCOL + CB - 1) // CB   # 17 mm blocks (last has 2 live columns)
NOBLK = OWC // CB   # 16 output blocks
COLP = NBLK * CB    # 272, padded column axis for uniform DMA slices
DT = mybir.dt.float32

_compiled_nc = None


def _build_nc():
    nc = bacc.Bacc("TRN2")

    xc_d = nc.dram_tensor("xc", [IC + 1, COLP, B], DT, kind="ExternalInput")
    wm1_d = nc.dram_tensor("wm1", [IC, COLP, 2 * OC], DT, kind="ExternalInput")
    wm2_d = nc.dram_tensor("wm2", [IC + 1, COLP, OC], DT, kind="ExternalInput")
    out_d = nc.dram_tensor("out", [OC, OWC, B], DT, kind="ExternalOutput")

    with tile.TileContext(nc) as tc:
        with (
            tc.tile_pool(name="w", bufs=4) as wpool,
            tc.tile_pool(name="x", bufs=4) as xpool,
            tc.tile_pool(name="o", bufs=3) as opool,
            tc.tile_pool(name="ps", bufs=4, space="PSUM") as pspool,
        ):
            wm1_t = [None] * NBLK
            wm2_t = [None] * NBLK
            xc_t = [None] * NBLK
            p1_t = [None] * NBLK
            p2_t = [None] * NBLK

            def load_block(bi):
                sl = slice(bi * CB, (bi + 1) * CB)
                wm1_t[bi] = wpool.tile([IC, CB, 2 * OC], DT, tag="wm1", name=f"wm1_{bi}")
                wm2_t[bi] = wpool.tile([IC + 1, CB, OC], DT, tag="wm2", name=f"wm2_{bi}")
                xc_t[bi] = xpool.tile([IC + 1, CB, B], DT, tag="xc", name=f"xc_{bi}")
                nc.sync.dma_start(out=wm1_t[bi][:], in_=wm1_d[:, sl, :])
                nc.sync.dma_start(out=wm2_t[bi][:], in_=wm2_d[:, sl, :])
                nc.sync.dma_start(out=xc_t[bi][:], in_=xc_d[:, sl, :])
                p1_t[bi] = pspool.tile([2 * OC, CB, B], DT, tag="p1", name=f"p1_{bi}")
                p2_t[bi] = pspool.tile([OC, CB, B], DT, tag="p2", name=f"p2_{bi}")

            for c in range(NCOL):
                bi, s = c // CB, c % CB
                if s == 0:
                    load_block(bi)
                # mm1: k0 of out[c] (rows 0:64) + k1 of out[c-1] (rows 64:128)
                nc.tensor.matmul(
                    p1_t[bi][:, s, :],
                    wm1_t[bi][:, s, :],
                    xc_t[bi][0:IC, s, :],
                    start=True,
                    stop=True,
                )
                # mm2: k2 + bias of out[c-2], into its own psum slot (indexed
                # by target position c-2; no PSUM accumulation — start=True
                # clears has_written bank-wide, so cross-column accumulation
                # in a shared bank silently overwrites)
                if c >= 2:
                    b2, s2 = (c - 2) // CB, (c - 2) % CB
                    nc.tensor.matmul(
                        p2_t[b2][:, s2, :],
                        wm2_t[bi][:, s, :],
                        xc_t[bi][:, s, :],
                        start=True,
                        stop=True,
                    )
                # after mm2(16b+17) the output block b is fully computed
                if c >= CB + 1 and (c - 1) % CB == 0:
                    ob = (c - 1) // CB - 1
                    combine_store(nc, opool, p1_t, p2_t, out_d, ob)
            combine_store(nc, opool, p1_t, p2_t, out_d, NOBLK - 1)

    nc.compile()
    return nc


def combine_store(nc, opool, p1_t, p2_t, out_d, ob):
    # out[w] = p1[0:64, slot w] (k0) + p1[64:128, slot w+1] (k1)
    #        + p2[slot w] (k2+bias).  DVE reads at most one PSUM operand per
    # op, so the k1 rows are staged to SBUF on ScalarE first.
    pa, pb = p1_t[ob], p1_t[ob + 1]
    kc = opool.tile([OC, CB, B], DT, tag="kc", name=f"kc_{ob}")
    nc.scalar.copy(out=kc[:, 0 : CB - 1, :], in_=pa[OC : 2 * OC, 1:CB, :])
    nc.scalar.copy(out=kc[:, CB - 1, :], in_=pb[OC : 2 * OC, 0, :])
    t = opool.tile([OC, CB, B], DT, tag="t", name=f"t_{ob}")
    nc.vector.tensor_add(out=t[:], in0=pa[0:OC, :, :], in1=kc[:])
    o = opool.tile([OC, CB, B], DT, tag="o", name=f"o_{ob}")
    nc.vector.tensor_add(out=o[:], in0=p2_t[ob][:], in1=t[:])
    nc.sync.dma_start(out=out_d[:, ob * CB : (ob + 1) * CB, :], in_=o[:])


def _get_nc():
    global _compiled_nc
    if _compiled_nc is None:
        _compiled_nc = _build_nc()
    return _compiled_nc


def shard_inputs(x, weights, bias):
    x = np.ascontiguousarray(np.asarray(x, dtype=np.float32))
    weights = np.asarray(weights, dtype=np.float32)
    bias = np.asarray(bias, dtype=np.float32)

    xp = np.pad(x, ((0, 0), (0, 0), (1, 1)))
    xpT = np.ascontiguousarray(xp.transpose(1, 2, 0))  # (IC, W+2, B)

    in_maps = []
    for c in range(NCORES):
        ws = c * OWC
        xc = np.zeros((IC + 1, COLP, B), np.float32)
        xc[0:IC, 0:NCOL, :] = xpT[:, ws : ws + NCOL, :]
        xc[IC, :, :] = 1.0

        wsl = weights[ws : ws + OWC]  # (OWC, OC, IC, KS)
        wT = wsl.transpose(3, 2, 0, 1)  # (KS, IC, OWC, OC)

        wm1 = np.zeros((IC, COLP, 2 * OC), np.float32)
        wm1[:, 0:OWC, 0:OC] = wT[0]          # k0 of out[c] at column c
        wm1[:, 1 : OWC + 1, OC : 2 * OC] = wT[1]  # k1 of out[c-1] at column c

        wm2 = np.zeros((IC + 1, COLP, OC), np.float32)
        wm2[0:IC, 2 : OWC + 2, :] = wT[2]    # k2 of out[c-2] at column c
        wm2[IC, 2 : OWC + 2, :] = bias[:, ws : ws + OWC].T

        in_maps.append(
            {
                "xc": xc,
                "wm1": np.ascontiguousarray(wm1),
                "wm2": np.ascontiguousarray(wm2),
            }
        )
    return in_maps


def run_sharded(x, weights, bias, trace=False):
    nc = _get_nc()
    in_maps = shard_inputs(x, weights, bias)
    res = run_bass_kernel_spmd(nc, in_maps, list(range(NCORES)), trace=trace)
    out = np.empty((B, OC, W), np.float32)
    for c in range(NCORES):
        out[:, :, c * OWC : (c + 1) * OWC] = res.results[c]["out"].transpose(2, 0, 1)
    return out, res


def kernel(x, weights, bias):
    out, _ = run_sharded(x, weights, bias)
    return out


# revision 9
# speedup vs baseline: 1.4966x; 1.4966x over previous
"""LocallyConnected1d Trainium2 kernel.

Problem: out[b, oc, w] = sum_{ic,k} xp[b, ic, w+k] * W[w, oc, ic, k] + bias[oc, w]
  x: (32, 64, 2048) f32, weights: (2048, 64, 64, 3) f32, bias: (64, 2048) f32
  out: (32, 64, 2048) f32.  xp = x padded by 1 on both sides of the last axis.

Sharding: output_width (2048) is split into 8 contiguous chunks of 256, one per
NeuronCore.  Weights dominate the traffic (100 MB) and are fully sharded this
way (12.6 MB/core); x is sent with a 2-column halo.

Per-core compute: for each position w the contraction over (ic, k) + bias is a
193-term dot product, done as two PSUM-accumulated matmuls:
  mm1: K=128 rows = (k=0, ic=0..63) ++ (k=1, ic=0..63),  lhsT=[128, 64oc], rhs=[128, 32b]
  mm2: K=65  rows = (k=2, ic=0..63) ++ bias row,         lhsT=[65, 64oc],  rhs=[65, 32b]
The bias is folded in as lhsT row 64 of mm2 against a constant ones row in rhs.

Host-side prep (numpy, cheap vs. the 100MB HBM traffic on device):
  wa[j, w, oc] = W[ws+w, oc, j%64, j//64]        j in [0,128)   (k-major)
  wb[j, w, oc] = W[ws+w, oc, j, 2] for j<64;  wb[64, w, oc] = bias[oc, ws+w]
  x1[j, c, b]  = xp[b, j%64, ws+c + j//64]       j in [0,128)
  x2[j, c, b]  = xp[b, j, ws+c+2] for j<64;   x2[64, c, b] = 1.0
"""

import numpy as np

import concourse.bacc as bacc
import concourse.mybir as mybir
import concourse.tile as tile
from concourse.bass_utils import run_bass_kernel_spmd

B, IC, OC, KS, W = 32, 64, 64, 3, 2048
NCORES = 8
OWC = W // NCORES  # 256 positions per core
CH = 16            # positions per chunk; psum tile = one bank
NCH = OWC // CH
DT = mybir.dt.float32

_compiled_nc = None


def _build_nc():
    nc = bacc.Bacc("TRN2")

    x1_d = nc.dram_tensor("x1", [2 * IC, OWC, B], DT, kind="ExternalInput")
    x2_d = nc.dram_tensor("x2", [IC + 1, OWC, B], DT, kind="ExternalInput")
    wa_d = nc.dram_tensor("wa", [2 * IC, OWC, OC], DT, kind="ExternalInput")
    wb_d = nc.dram_tensor("wb", [IC + 1, OWC, OC], DT, kind="ExternalInput")
    out_d = nc.dram_tensor("out", [OC, OWC, B], DT, kind="ExternalOutput")

    with tile.TileContext(nc) as tc:
        with (
            tc.tile_pool(name="w", bufs=4) as wpool,
            tc.tile_pool(name="x", bufs=4) as xpool,
            tc.tile_pool(name="o", bufs=3) as opool,
            tc.tile_pool(name="ps", bufs=4, space="PSUM") as pspool,
        ):
            for ci in range(NCH):
                sl = slice(ci * CH, (ci + 1) * CH)
                wa = wpool.tile([2 * IC, CH, OC], DT, tag="wa")
                wb = wpool.tile([IC + 1, CH, OC], DT, tag="wb")
                x1 = xpool.tile([2 * IC, CH, B], DT, tag="x1")
                x2 = xpool.tile([IC + 1, CH, B], DT, tag="x2")
                nc.sync.dma_start(out=wa[:], in_=wa_d[:, sl, :])
                nc.sync.dma_start(out=wb[:], in_=wb_d[:, sl, :])
                nc.sync.dma_start(out=x1[:], in_=x1_d[:, sl, :])
                nc.sync.dma_start(out=x2[:], in_=x2_d[:, sl, :])

                ps = pspool.tile([OC, CH, B], DT, tag="ps")
                for w in range(CH):
                    nc.tensor.matmul(
                        ps[:, w, :],
                        wa[:, w, :],
                        x1[:, w, :],
                        start=True,
                        stop=False,
                    )
                    nc.tensor.matmul(
                        ps[:, w, :],
                        wb[:, w, :],
                        x2[:, w, :],
                        start=False,
                        stop=True,
                    )

                ob = opool.tile([OC, CH, B], DT, tag="ob")
                nc.scalar.copy(out=ob[:], in_=ps[:])
                nc.sync.dma_start(out=out_d[:, sl, :], in_=ob[:])

    nc.compile()
    return nc


def _get_nc():
    global _compiled_nc
    if _compiled_nc is None:
        _compiled_nc = _build_nc()
    return _compiled_nc


def shard_inputs(x, weights, bias):
    x = np.ascontiguousarray(np.asarray(x, dtype=np.float32))
    weights = np.asarray(weights, dtype=np.float32)
    bias = np.asarray(bias, dtype=np.float32)

    xp = np.pad(x, ((0, 0), (0, 0), (1, 1)))
    xpT = np.ascontiguousarray(xp.transpose(1, 2, 0))  # (IC, W+2, B)
    ones = np.ones((1, OWC, B), np.float32)

    in_maps = []
    for c in range(NCORES):
        ws = c * OWC
        x1 = np.concatenate(
            [xpT[:, ws : ws + OWC, :], xpT[:, ws + 1 : ws + 1 + OWC, :]], axis=0
        )
        x2 = np.concatenate([xpT[:, ws + 2 : ws + 2 + OWC, :], ones], axis=0)
        wsl = weights[ws : ws + OWC]  # (OWC, OC, IC, KS)
        wa = np.ascontiguousarray(wsl[:, :, :, 0:2].transpose(3, 2, 0, 1)).reshape(
            2 * IC, OWC, OC
        )
        wb = np.concatenate(
            [wsl[:, :, :, 2].transpose(2, 0, 1), bias[:, ws : ws + OWC].T[None]],
            axis=0,
        )
        in_maps.append(
            {
                "x1": np.ascontiguousarray(x1),
                "x2": np.ascontiguousarray(x2),
                "wa": np.ascontiguousarray(wa),
                "wb": np.ascontiguousarray(wb),
            }
        )
    return in_maps


def run_sharded(x, weights, bias, trace=False):
    nc = _get_nc()
    in_maps = shard_inputs(x, weights, bias)
    res = run_bass_kernel_spmd(nc, in_maps, list(range(NCORES)), trace=trace)
    out = np.empty((B, OC, W), np.float32)
    for c in range(NCORES):
        out[:, :, c * OWC : (c + 1) * OWC] = res.results[c]["out"].transpose(2, 0, 1)
    return out, res


def kernel(x, weights, bias):
    out, _ = run_sharded(x, weights, bias)
    return out


# revision 10
# speedup vs baseline: 1.7789x; 1.1886x over previous
"""LocallyConnected1d Trainium2 kernel.

Problem: out[b, oc, w] = sum_{ic,k} xp[b, ic, w+k] * W[w, oc, ic, k] + bias[oc, w]
  x: (32, 64, 2048) f32, weights: (2048, 64, 64, 3) f32, bias: (64, 2048) f32
  out: (32, 64, 2048) f32.  xp = x padded by 1 on both sides of the last axis.

Sharding: output_width (2048) is split into 8 contiguous chunks of 256, one per
NeuronCore.  Weights dominate the traffic (100 MB) and are fully sharded this
way (12.6 MB/core); x is sent with a 2-column halo.

Per-core compute: for each position w the contraction over (ic, k) + bias is a
193-term dot product, done as two PSUM-accumulated fp32 matmuls:
  mm1: K=128 rows = (k=0, ic=0..63) ++ (k=1, ic=0..63),  lhsT=[128, 64oc], rhs=[128, 32b]
  mm2: K=65  rows = (k=2, ic=0..63) ++ bias row,         lhsT=[65, 64oc],  rhs=[65, 32b]
The bias is folded in as lhsT row 64 of mm2 against a constant ones row in rhs.

fp32 matmuls lower to 2 HW passes (LDW+MM each); with N=32 the MM pass costs
N*4 = 128 PE cycles, so the PE floor is ~512 cyc/position at the observed
1.2 GHz clock (~110 us/core).  DMA (21 MB/core) is packet-rate-bound, so
weights/x are fetched in fat 64-position slices (4-16 KB contiguous per
partition) while PSUM/compute runs in 16-position chunks (1 bank each).

Host-side prep (numpy, cheap vs. the 100MB HBM traffic on device):
  wa[j, w, oc] = W[ws+w, oc, j%64, j//64]        j in [0,128)   (k-major)
  wb[j, w, oc] = W[ws+w, oc, j, 2] for j<64;  wb[64, w, oc] = bias[oc, ws+w]
  x1[j, c, b]  = xp[b, j%64, ws+c + j//64]       j in [0,128)
  x2[j, c, b]  = xp[b, j, ws+c+2] for j<64;   x2[64, c, b] = 1.0
"""

import numpy as np

import concourse.bacc as bacc
import concourse.mybir as mybir
import concourse.tile as tile
from concourse.bass_utils import run_bass_kernel_spmd

B, IC, OC, KS, W = 32, 64, 64, 3, 2048
NCORES = 8
OWC = W // NCORES  # 256 positions per core
CH = 16            # compute chunk; psum tile = [64, CH*32] = one bank
DCH = 64           # DMA chunk (positions per weight/x fetch)
DT = mybir.dt.float32

_compiled_nc = None


def _build_nc():
    nc = bacc.Bacc("TRN2")

    x1_d = nc.dram_tensor("x1", [2 * IC, OWC, B], DT, kind="ExternalInput")
    x2_d = nc.dram_tensor("x2", [IC + 1, OWC, B], DT, kind="ExternalInput")
    wa_d = nc.dram_tensor("wa", [2 * IC, OWC, OC], DT, kind="ExternalInput")
    wb_d = nc.dram_tensor("wb", [IC + 1, OWC, OC], DT, kind="ExternalInput")
    out_d = nc.dram_tensor("out", [OC, OWC, B], DT, kind="ExternalOutput")

    # First DMA slice is small so the PE starts quickly; the rest are fat.
    dma_slices = [(0, CH), (CH, DCH - CH)]
    p = DCH
    while p < OWC:
        dma_slices.append((p, DCH))
        p += DCH

    with tile.TileContext(nc) as tc:
        with (
            tc.tile_pool(name="w", bufs=2) as wpool,
            tc.tile_pool(name="x", bufs=2) as xpool,
            tc.tile_pool(name="o", bufs=3) as opool,
            tc.tile_pool(name="ps", bufs=4, space="PSUM") as pspool,
        ):
            loaded = []  # (start, len, wa, wb, x1, x2)
            for si, (p0, plen) in enumerate(dma_slices):
                sl = slice(p0, p0 + plen)
                wa = wpool.tile([2 * IC, plen, OC], DT, tag="wa", name=f"wa_{si}")
                wb = wpool.tile([IC + 1, plen, OC], DT, tag="wb", name=f"wb_{si}")
                x1 = xpool.tile([2 * IC, plen, B], DT, tag="x1", name=f"x1_{si}")
                x2 = xpool.tile([IC + 1, plen, B], DT, tag="x2", name=f"x2_{si}")
                nc.sync.dma_start(out=wa[:], in_=wa_d[:, sl, :])
                nc.sync.dma_start(out=wb[:], in_=wb_d[:, sl, :])
                nc.sync.dma_start(out=x1[:], in_=x1_d[:, sl, :])
                nc.sync.dma_start(out=x2[:], in_=x2_d[:, sl, :])
                loaded.append((p0, plen, wa, wb, x1, x2))

            for p0, plen, wa, wb, x1, x2 in loaded:
                for c0 in range(0, plen, CH):
                    ps = pspool.tile([OC, CH, B], DT, tag="ps", name=f"ps_{p0 + c0}")
                    for w in range(CH):
                        wl = c0 + w
                        nc.tensor.matmul(
                            ps[:, w, :],
                            wa[:, wl, :],
                            x1[:, wl, :],
                            start=True,
                            stop=False,
                        )
                        nc.tensor.matmul(
                            ps[:, w, :],
                            wb[:, wl, :],
                            x2[:, wl, :],
                            start=False,
                            stop=True,
                        )
                    ob = opool.tile([OC, CH, B], DT, tag="ob", name=f"ob_{p0 + c0}")
                    nc.scalar.copy(out=ob[:], in_=ps[:])
                    nc.sync.dma_start(
                        out=out_d[:, p0 + c0 : p0 + c0 + CH, :], in_=ob[:]
                    )

    nc.compile()
    return nc


def _get_nc():
    global _compiled_nc
    if _compiled_nc is None:
        _compiled_nc = _build_nc()
    return _compiled_nc


def shard_inputs(x, weights, bias):
    x = np.ascontiguousarray(np.asarray(x, dtype=np.float32))
    weights = np.asarray(weights, dtype=np.float32)
    bias = np.asarray(bias, dtype=np.float32)

    xp = np.pad(x, ((0, 0), (0, 0), (1, 1)))
    xpT = np.ascontiguousarray(xp.transpose(1, 2, 0))  # (IC, W+2, B)
    ones = np.ones((1, OWC, B), np.float32)

    in_maps = []
    for c in range(NCORES):
        ws = c * OWC
        x1 = np.concatenate(
            [xpT[:, ws : ws + OWC, :], xpT[:, ws + 1 : ws + 1 + OWC, :]], axis=0
        )
        x2 = np.concatenate([xpT[:, ws + 2 : ws + 2 + OWC, :], ones], axis=0)
        wsl = weights[ws : ws + OWC]  # (OWC, OC, IC, KS)
        wa = np.ascontiguousarray(wsl[:, :, :, 0:2].transpose(3, 2, 0, 1)).reshape(
            2 * IC, OWC, OC
        )
        wb = np.concatenate(
            [wsl[:, :, :, 2].transpose(2, 0, 1), bias[:, ws : ws + OWC].T[None]],
            axis=0,
        )
        in_maps.append(
            {
                "x1": np.ascontiguousarray(x1),
                "x2": np.ascontiguousarray(x2),
                "wa": np.ascontiguousarray(wa),
                "wb": np.ascontiguousarray(wb),
            }
        )
    return in_maps


def run_sharded(x, weights, bias, trace=False):
    nc = _get_nc()
    in_maps = shard_inputs(x, weights, bias)
    res = run_bass_kernel_spmd(nc, in_maps, list(range(NCORES)), trace=trace)
    out = np.empty((B, OC, W), np.float32)
    for c in range(NCORES):
        out[:, :, c * OWC : (c + 1) * OWC] = res.results[c]["out"].transpose(2, 0, 1)
    return out, res


def kernel(x, weights, bias):
    out, _ = run_sharded(x, weights, bias)
    return out
